# revision 14
# baseline (speedup 1.0000x reference)
"""Causal multi-head self-attention on 8 Trainium2 NeuronCores (v2).

Problem (hardcoded): x [4, 2048, 1024] fp32, w_qkv [3072, 1024], w_out
[1024, 1024], token_positions [2048] int32; H=16 heads, Dh=64, RoPE
(interleaved pairs, theta=10000), causal softmax, output projection.

Sharding: 8 cores = 4 batches x 2 head-groups (8 heads each). Each core
computes qkv projection for its heads, RoPE, causal attention, and a
partial output projection over its 512 y-features (fp16 partials summed
on host).

v2 changes vs v1:
  - fp16 matmul operands everywhere (q/k/v/exp/y/weights): FWL halves
    LDWEIGHTS, 16-bit DVE ops run 2x, SBUF traffic halves.
  - One fused schedule: QKV-projection matmul groups for chunk c+1 and
    out-projection groups are interleaved as PE filler work inside the
    attention tile stream of chunk c, so ScalarE exp (the attention
    pacer) hides behind PE work and the PE never idles (HAM stays at
    K=8/8).
  - Attention per (c,hp): scores row-tiled (2 heads concurrent, K=64 at
    base partitions 0/64), exp on ScalarE (fp16 out), diagonal tri-mask
    on DVE in fp16, PV accumulates both heads into one [65,1024] psum
    pair; normalize = one batched reciprocal + gpsimd broadcast + 2 DVE
    muls.
  - PSUM: aux pool (QKV + out-proj, 2 banks) + scores (4) + pv (2) = 8.
"""

import math

import numpy as np

import concourse.bacc as bacc
import concourse.mybir as mybir
import concourse.tile as tile
from concourse.bass_utils import run_bass_kernel_spmd

F32 = mybir.dt.float32
F16 = mybir.dt.float16
F32R = mybir.dt.float32r

B, S, D = 4, 2048, 1024
H = 16
DH = 64
H_CORE = 8          # heads per core
N_CORES = 8
ROPE_THETA = 10000.0

CH = 512            # seq chunk (free dim of most matmuls)
N_CHUNKS = S // CH          # 4
N_STILES = S // 128         # 16
N_DTILES = D // 128         # 8
VSLOT = 66          # v_ext slot stride (65 used + 1 pad for 4B alignment)
SWAP_MASK = [i ^ 1 for i in range(32)]

_EXP = mybir.ActivationFunctionType.Exp
_MUL = None  # filled lazily (AluOpType)


def build_nc():
    nc = bacc.Bacc("TRN2", target_bir_lowering=False, debug=False)

    xT = nc.dram_tensor("xT", [D, S], F16, kind="ExternalInput").ap()
    # [d, f] with f = [q-heads (512) | k-heads (512)] for this core's 8 heads
    wqkT = nc.dram_tensor("wqkT", [D, 2 * H_CORE * DH], F16, kind="ExternalInput").ap()
    wvT = nc.dram_tensor("wvT", [D, H_CORE * DH], F16, kind="ExternalInput").ap()
    woT = nc.dram_tensor("woT", [H_CORE * DH, D], F16, kind="ExternalInput").ap()
    cosT = nc.dram_tensor("cosT", [128, S], F32, kind="ExternalInput").ap()
    sinT = nc.dram_tensor("sinT", [128, S], F32, kind="ExternalInput").ap()
    # [tri x4]: tri[i, j] = 1 if i <= j else 0 (fp16)
    trimask = nc.dram_tensor("trimask", [128, 512], F16, kind="ExternalInput").ap()
    outT = nc.dram_tensor("outT", [D, S], F16, kind="ExternalOutput").ap()

    with tile.TileContext(nc) as tc:
        _build_body(nc, tc, xT, wqkT, wvT, woT, cosT, sinT, trimask, outT)
    nc.compile()
    return nc


def _build_body(nc, tc, xT, wqkT, wvT, woT, cosT, sinT, trimask, outT):
    from concourse.alu_op_type import AluOpType
    mul_op = AluOpType.mult

    with tc.tile_pool(name="persist", bufs=1) as persist, \
         tc.tile_pool(name="qkv", bufs=1) as qkv_pool, \
         tc.tile_pool(name="w", bufs=1) as w_pool, \
         tc.tile_pool(name="xch", bufs=2) as xch_pool, \
         tc.tile_pool(name="rtmp", bufs=3) as rtmp_pool, \
         tc.tile_pool(name="exp", bufs=4) as exp_pool, \
         tc.tile_pool(name="sm", bufs=2) as sm_pool, \
         tc.tile_pool(name="p3t", bufs=2) as p3_pool, \
         tc.tile_pool(name="psaux", bufs=2, space="PSUM") as ps_aux, \
         tc.tile_pool(name="pss", bufs=2, space="PSUM") as ps_s_pool, \
         tc.tile_pool(name="pspv", bufs=1, space="PSUM") as ps_pv_pool:

        cos_sb = persist.tile([128, S], F32, tag="cos")
        sin_sb = persist.tile([128, S], F32, tag="sin")
        tri_sb = persist.tile([128, 512], F16, tag="tri")

        # all 8 heads: per-(pair, chunk) fp16 tiles for q, k; v_ext s-tiles
        q_rot = [[qkv_pool.tile([128, CH], F16, tag=f"q{i}_{c}",
                                name=f"qrot{i}_{c}")
                  for c in range(N_CHUNKS)] for i in range(4)]
        k_rot = [[qkv_pool.tile([128, CH], F16, tag=f"k{i}_{c}",
                                name=f"krot{i}_{c}")
                  for c in range(N_CHUNKS)] for i in range(4)]
        v_ext = [qkv_pool.tile([128, H_CORE * VSLOT], F16, tag=f"v{i}",
                               name=f"vext{i}")
                 for i in range(N_STILES)]
        yT = [qkv_pool.tile([128, S], F16, tag=f"yT{i}", name=f"yT{i}")
              for i in range(4)]

        ones_sm = qkv_pool.tile([128, H_CORE], F16, tag="ones1", name="ones_sm")
        nc.vector.memset(ones_sm[:], 1.0)
        warm = qkv_pool.tile([128, H_CORE], F32, tag="warm", name="warm_sm")
        nc.scalar.activation(warm[:], ones_sm[:], _EXP, scale=1.0)
        # junk matmuls keep the PE busy while the first DMAs land (HAM warm)
        junk_f = qkv_pool.tile([128, 512], F32, tag="junkf", name="junkf_sm")
        nc.vector.memset(junk_f[:], 1.0)
        junk = qkv_pool.tile([128, 512], F16, tag="junk", name="junk_sm")
        nc.vector.tensor_copy(junk[:], junk_f[:])
        for st in range(N_STILES):
            nc.vector.tensor_copy(
                v_ext[st][:].rearrange("p (h e) -> p h e", e=VSLOT)[:, :, 64:65],
                ones_sm[:].unsqueeze(2))

        # ---- DMAs: weights first, then x chunk 0; tables afterwards ----
        wqk_sb = []
        for dt in range(N_DTILES):
            w = w_pool.tile([128, 1024], F16, tag=f"wqk{dt}", name=f"wqk{dt}")
            nc.sync.dma_start(w[:], wqkT[128 * dt:128 * (dt + 1), :])
            wqk_sb.append(w)
        x_chunks = [None] * N_CHUNKS

        def dma_x_chunk(c):
            ts = []
            for dt in range(N_DTILES):
                t = xch_pool.tile([128, CH], F16, tag=f"xc{dt}",
                                  name=f"xch{c}_{dt}")
                nc.sync.dma_start(
                    t[:], xT[128 * dt:128 * (dt + 1), CH * c:CH * (c + 1)])
                ts.append(t)
            x_chunks[c] = ts

        dma_x_chunk(0)
        wv_sb = []
        for dt in range(N_DTILES):
            w = w_pool.tile([128, 512], F16, tag=f"wv{dt}", name=f"wv{dt}")
            nc.sync.dma_start(w[:], wvT[128 * dt:128 * (dt + 1), :])
            wv_sb.append(w)
        dma_x_chunk(1)
        wo_sb = []
        for dt in range(4):
            w = w_pool.tile([128, D], F16, tag=f"wo{dt}", name=f"wo{dt}")
            nc.sync.dma_start(w[:], woT[128 * dt:128 * (dt + 1), :])
            wo_sb.append(w)
        nc.sync.dma_start(cos_sb[:], cosT)
        nc.sync.dma_start(sin_sb[:], sinT)
        nc.sync.dma_start(tri_sb[:], trimask)

        # HAM warm-up while DMAs land
        ps_warm = ps_aux.tile([128, 512], F32, tag="aux")
        for i in range(24):
            nc.tensor.matmul(ps_warm[:], junk[:, 0:128], junk[:],
                             start=(i == 0), stop=(i == 23))
        nc.vector.tensor_copy(warm[0:1, 0:1], ps_warm[0:1, 0:1])

        # ---- filler group emitters ----
        def emit_qk_group(c, ft):
            """QK projection f-tile ft (0-3 q pairs, 4-7 k pairs) + rope."""
            cs = slice(CH * c, CH * (c + 1))
            dest = q_rot[ft][c] if ft < 4 else k_rot[ft - 4][c]
            ps_qk = ps_aux.tile([128, CH], F32, tag="aux")
            for dt in range(N_DTILES):
                nc.tensor.matmul(
                    ps_qk[:],
                    wqk_sb[dt][:, 128 * ft:128 * (ft + 1)],
                    x_chunks[c][dt][:],
                    start=(dt == 0), stop=(dt == N_DTILES - 1),
                )
            shuf = rtmp_pool.tile([128, CH], F32, tag="shuf")
            nc.vector.stream_shuffle(shuf[:], ps_qk[:], SWAP_MASK)
            t1 = rtmp_pool.tile([128, CH], F16, tag="t1")
            nc.vector.tensor_mul(t1[:], ps_qk[:], cos_sb[:, cs])
            t2 = rtmp_pool.tile([128, CH], F16, tag="t2")
            nc.gpsimd.tensor_mul(t2[:], shuf[:], sin_sb[:, cs])
            nc.gpsimd.tensor_add(dest[:], t1[:], t2[:])

        def emit_v_group(c, stl):
            """V projection s-tile (4c+stl) + strided copy into v_ext."""
            st = 4 * c + stl
            ps_v = ps_aux.tile([128, 512], F32, tag="aux")
            for dt in range(N_DTILES):
                nc.tensor.matmul(
                    ps_v[:],
                    x_chunks[c][dt][:, 128 * stl:128 * (stl + 1)],
                    wv_sb[dt][:],
                    start=(dt == 0), stop=(dt == N_DTILES - 1),
                )
            out_ap = v_ext[st][:].rearrange(
                "p (h e) -> p h e", e=VSLOT)[:, :, 0:64]
            in_ap = ps_v[:].rearrange("p (h e) -> p h e", e=64)
            nc.vector.tensor_copy(out_ap, in_ap)

        def emit_p3_group(pc, ot):
            """Out-projection o-tile ot for seq chunk pc."""
            ps_o = ps_aux.tile([128, CH], F32, tag="aux")
            for dt in range(4):
                nc.tensor.matmul(
                    ps_o[:],
                    wo_sb[dt][:, 128 * ot:128 * (ot + 1)],
                    yT[dt][:, CH * pc:CH * (pc + 1)],
                    start=(dt == 0), stop=(dt == 3),
                )
            osb = p3_pool.tile([128, CH], F16, tag="osb")
            nc.scalar.copy(osb[:], ps_o[:])
            nc.sync.dma_start(
                outT[128 * ot:128 * (ot + 1), CH * pc:CH * (pc + 1)],
                osb[:])

        # ---- QKV chunk 0 up front (QK first: rope latency) ----
        for ft in range(8):
            emit_qk_group(0, ft)
            if ft == 0:
                dma_x_chunk(1)      # prefetch chunk 1 early
        for stl in range(4):
            emit_v_group(0, stl)

        # ---- fused attention + fillers ----
        scale = 1.0 / math.sqrt(DH)

        def att_chunk(c, fillers):
            """Attention for chunk c (4 head pairs), popping filler
            emitters between tiles. fillers: list of zero-arg closures."""
            nt = 4 * c + 4
            n_tiles = 4 * nt
            # spread fillers evenly across all tiles of this chunk
            acc = 0.0
            step = len(fillers) / max(1, n_tiles)
            fi = 0
            tile_idx = 0

            for hp in range(4):
                pv = ps_pv_pool.tile([65, 1024], F32, tag="pv")
                qt = q_rot[hp][c]
                ets = [None] * nt

                def emit_scores(t):
                    r = t - 4 * c
                    coff = 128 * r if r > 0 else 0
                    ps = ps_s_pool.tile([128, 2 * CH], F32, tag="ps_s")
                    kt = k_rot[hp][t // 4]
                    ks = slice(128 * (t % 4), 128 * (t % 4 + 1))
                    qs = slice(coff, CH)
                    nc.tensor.matmul(
                        ps[:, coff:CH], kt[0:64, ks], qt[0:64, qs],
                        start=True, stop=True)
                    nc.tensor.matmul(
                        ps[:, CH + coff:2 * CH], kt[64:128, ks], qt[64:128, qs],
                        start=True, stop=True)
                    et = exp_pool.tile([128, 2 * CH], F16, tag="et")
                    src = ps[:].rearrange("p (b n) -> p b n", b=2)[:, :, coff:CH]
                    dst = et[:].rearrange("p (b n) -> p b n", b=2)[:, :, coff:CH]
                    nc.scalar.activation(dst, src, _EXP, scale=scale)
                    if r >= 0:
                        dg = et[:].rearrange("p (b n) -> p b n", b=2)[
                            :, :, coff:coff + 128]
                        nc.vector.tensor_mul(
                            dg, dg,
                            tri_sb[:, 0:256].rearrange("p (b n) -> p b n", b=2))
                    ets[t] = et

                def emit_pv(t):
                    r = t - 4 * c
                    coff = 128 * r if r > 0 else 0
                    et = ets[t]
                    for hl in range(2):
                        hcol = (2 * hp + hl) * VSLOT
                        nc.tensor.matmul(
                            pv[:, CH * hl + coff:CH * hl + CH],
                            v_ext[t][:, hcol:hcol + 65],
                            et[:, CH * hl + coff:CH * hl + CH],
                            start=(t == 0), stop=(t == nt - 1),
                        )

                emit_scores(0)
                for t in range(nt):
                    if t + 1 < nt:
                        emit_scores(t + 1)
                    # filler slot
                    acc += step
                    while fi < len(fillers) and acc >= 1.0:
                        fillers[fi]()
                        fi += 1
                        acc -= 1.0
                    tile_idx += 1
                    emit_pv(t)

                # normalize: one copy frees the pv psum fast (the next hp's
                # PV WAR-waits on it); recip/broadcast/muls run from SBUF
                # off the PE critical path.
                sm = sm_pool.tile([1, 1024], F32, tag="sm")
                nc.vector.tensor_copy(sm[:], pv[64:65, :])
                yu = sm_pool.tile([64, 1024], F32, tag="yu")
                nc.vector.tensor_copy(yu[:], pv[0:64, :])
                rc = sm_pool.tile([1, 1024], F32, tag="rc")
                nc.vector.reciprocal_approx_fast(rc[:], sm[:])
                bc = sm_pool.tile([64, 1024], F32, tag="bc")
                nc.gpsimd.partition_broadcast(bc[:], rc[:])
                cs = slice(CH * c, CH * (c + 1))
                nc.gpsimd.tensor_mul(
                    yT[hp][0:64, cs], yu[:, 0:CH], bc[:, 0:CH])
                nc.gpsimd.tensor_mul(
                    yT[hp][64:128, cs], yu[:, CH:2 * CH], bc[:, CH:2 * CH])
            # drain leftover fillers
            while fi < len(fillers):
                fillers[fi]()
                fi += 1

        for c in range(N_CHUNKS):
            fillers = []
            if c + 1 < N_CHUNKS:
                if c + 2 < N_CHUNKS:
                    fillers.append(lambda cc=c + 2: dma_x_chunk(cc))
                fillers += [lambda cc=c + 1, f=ft: emit_qk_group(cc, f)
                            for ft in range(8)]
                fillers += [lambda cc=c + 1, s=stl: emit_v_group(cc, s)
                            for stl in range(4)]
            else:
                for pc in range(3):
                    fillers += [lambda p=pc, o=ot: emit_p3_group(p, o)
                                for ot in range(8)]
            att_chunk(c, fillers)

        for ot in range(8):
            emit_p3_group(3, ot)


# ---------------------------------------------------------------------------
# Host side
# ---------------------------------------------------------------------------

_NC_CACHE = None


def _get_nc():
    global _NC_CACHE
    if _NC_CACHE is None:
        _NC_CACHE = build_nc()
    return _NC_CACHE


def _host_prep(x, w_qkv, w_out, token_positions):
    """Build the 8 per-core input maps."""
    x = np.ascontiguousarray(np.asarray(x, dtype=np.float32))
    w_qkv = np.asarray(w_qkv, dtype=np.float32)
    w_out = np.asarray(w_out, dtype=np.float32)
    pos = np.asarray(token_positions).astype(np.float32)

    half = DH // 2
    inv_freq = (1.0 / (ROPE_THETA ** (np.arange(half, dtype=np.float32) * (2.0 / DH))))
    ang = pos[:, None] * inv_freq[None, :]          # [S, 32]
    cos = np.cos(ang).astype(np.float32)            # [S, 32]
    sin = np.sin(ang).astype(np.float32)
    # [Dh, S] interleaved-pair layout, duplicated for 2 heads per tile
    cos64 = np.repeat(cos.T, 2, axis=0)             # [64, S]
    sin64 = np.repeat(sin.T, 2, axis=0)
    sgn = np.where(np.arange(DH) % 2 == 0, -1.0, 1.0).astype(np.float32)
    sinp = sin64 * sgn[:, None]
    cosT = np.ascontiguousarray(np.tile(cos64, (2, 1)))      # [128, S]
    sinT = np.ascontiguousarray(np.tile(sinp, (2, 1)))

    tri = np.triu(np.ones((128, 128), dtype=np.float16))     # keep i <= j
    trimask = np.ascontiguousarray(np.concatenate([tri] * 4, axis=1))

    wq, wk, wv = w_qkv[0:D], w_qkv[D:2 * D], w_qkv[2 * D:3 * D]

    in_maps = []
    for core in range(N_CORES):
        b, g = divmod(core, 2)
        rows = slice(512 * g, 512 * (g + 1))
        wqkT = np.ascontiguousarray(
            np.concatenate([wq[rows], wk[rows]], axis=0).T.astype(np.float16))
        wvT = np.ascontiguousarray(wv[rows].T.astype(np.float16))
        woT = np.ascontiguousarray(w_out[:, rows].T.astype(np.float16))
        xT = np.ascontiguousarray(x[b].T.astype(np.float16))
        in_maps.append({
            "xT": xT, "wqkT": wqkT, "wvT": wvT, "woT": woT,
            "cosT": cosT, "sinT": sinT, "trimask": trimask,
        })
    return in_maps


def _gather(results):
    out = np.empty((B, S, D), dtype=np.float32)
    for b in range(B):
        acc = (results[2 * b]["outT"].astype(np.float32)
               + results[2 * b + 1]["outT"].astype(np.float32))   # [D, S]
        out[b] = acc.T
    return out


def kernel(x, w_qkv, w_out, token_positions, _trace=False, _trace_kwargs=None):
    nc = _get_nc()
    in_maps = _host_prep(x, w_qkv, w_out, token_positions)
    kw = {}
    if _trace:
        kw["trace"] = True
        kw.update(_trace_kwargs or {})
    res = run_bass_kernel_spmd(nc, in_maps, list(range(N_CORES)), **kw)
    out = _gather(res.results)
    if _trace:
        return out, res
    return out
